# revision 1
# baseline (speedup 1.0000x reference)
"""GroupedQueryAttention distributed across 8 NeuronCores.

Sharding: data-parallel over batch (2) x sequence-row-parallel (4) per batch.
Each core computes K/V for its batch redundantly (cheap), Q/attention/output
projection only for its row block, so every core's output rows are complete
and the host only concatenates -- no collectives needed.

Falls back to a pure-numpy implementation if device execution fails.
"""
import numpy as np

D_MODEL = 2048
HQ = 16
HKV = 4
HEAD_DIM = 128
GROUP = 4
B, S = 2, 2048
RMS_EPS = 1.1920929e-07
ROPE_THETA = 10000.0
N_CORES = 8
ROWS_PER_CORE = S // 4  # 4 row blocks per batch


def _np_rmsnorm(x, w):
    var = np.mean(np.square(x), axis=-1, keepdims=True)
    return x * (1.0 / np.sqrt(var + RMS_EPS)) * w


def _np_rope(x, positions):
    # x: [..., s, d] interleaved pairs
    half = x.shape[-1] // 2
    inv_freq = 1.0 / (ROPE_THETA ** (np.arange(half, dtype=np.float32) / half))
    ang = positions.astype(np.float32)[:, None] * inv_freq[None, :]
    cos = np.cos(ang)
    sin = np.sin(ang)
    while cos.ndim < x.ndim:
        cos = cos[None]
        sin = sin[None]
    x1 = x[..., 0::2]
    x2 = x[..., 1::2]
    r1 = x1 * cos - x2 * sin
    r2 = x1 * sin + x2 * cos
    out = np.empty_like(x)
    out[..., 0::2] = r1
    out[..., 1::2] = r2
    return out


def _rows_block(x_b, row_lo, row_hi, Wq, bq, Wk, bk, Wv, bv, Wo, bo,
                qn_w, kn_w, gate_logits, mask, start_pos):
    """Compute output rows [row_lo:row_hi] for one batch, full heads."""
    ext = row_hi  # causal: keys needed only up to row_hi
    positions_q = start_pos + np.arange(row_lo, row_hi)
    positions_k = start_pos + np.arange(ext)

    xq = x_b[row_lo:row_hi]                       # [R, D]
    xk = x_b[:ext]                                # [ext, D]

    q = _np_rmsnorm(xq @ Wq + bq, qn_w)           # [R, D]
    k = _np_rmsnorm(xk @ Wk + bk, kn_w)           # [ext, 512]
    v = xk @ Wv + bv                              # [ext, 512]

    R = row_hi - row_lo
    q = q.reshape(R, HQ, HEAD_DIM).transpose(1, 0, 2)      # [hq, R, d]
    k = k.reshape(ext, HKV, HEAD_DIM).transpose(1, 0, 2)   # [hkv, ext, d]
    v = v.reshape(ext, HKV, HEAD_DIM).transpose(1, 0, 2)

    q = _np_rope(q, positions_q)
    k = _np_rope(k, positions_k)

    scale = 1.0 / np.sqrt(np.float32(HEAD_DIM))
    gates = 1.0 / (1.0 + np.exp(-gate_logits.astype(np.float32)))  # [HQ]
    m = mask[row_lo:row_hi, :ext]                 # [R, ext]

    attn_heads = np.empty((R, HQ, HEAD_DIM), dtype=np.float32)
    for g in range(HKV):
        kg = k[g]                                  # [ext, d]
        vg = v[g]
        for j in range(GROUP):
            h = g * GROUP + j
            s = (q[h] @ kg.T) * scale              # [R, ext]
            s = np.where(m, s, -np.inf).astype(np.float32)
            s -= s.max(axis=-1, keepdims=True)
            p = np.exp(s)
            p /= p.sum(axis=-1, keepdims=True)
            attn_heads[:, h, :] = (p @ vg) * gates[h]

    attn = attn_heads.reshape(R, D_MODEL)
    return (attn @ Wo + bo).astype(np.float32)


def _device_fn_factory():
    import jax
    import jax.numpy as jnp

    def f(xq, xk, pos_q, pos_k, Wq, bq, Wk, bk, Wv, bv, Wo, bo,
          qn_w, kn_w, gate_logits, m):
        def rms(t, w):
            var = jnp.mean(jnp.square(t), axis=-1, keepdims=True)
            return t * jax.lax.rsqrt(var + RMS_EPS) * w

        def rope(t, positions):
            half = t.shape[-1] // 2
            inv_freq = 1.0 / (ROPE_THETA ** (jnp.arange(half, dtype=jnp.float32) / half))
            ang = positions.astype(jnp.float32)[:, None] * inv_freq[None, :]
            cos = jnp.cos(ang)[None]
            sin = jnp.sin(ang)[None]
            x1 = t[..., 0::2]
            x2 = t[..., 1::2]
            r1 = x1 * cos - x2 * sin
            r2 = x1 * sin + x2 * cos
            return jnp.stack([r1, r2], axis=-1).reshape(t.shape)

        R = xq.shape[0]
        ext = xk.shape[0]
        q = rms(xq @ Wq + bq, qn_w)
        k = rms(xk @ Wk + bk, kn_w)
        v = xk @ Wv + bv
        q = q.reshape(R, HQ, HEAD_DIM).transpose(1, 0, 2)
        k = k.reshape(ext, HKV, HEAD_DIM).transpose(1, 0, 2)
        v = v.reshape(ext, HKV, HEAD_DIM).transpose(1, 0, 2)
        q = rope(q, pos_q)
        k = rope(k, pos_k)
        qg = q.reshape(HKV, GROUP, R, HEAD_DIM)
        scale = 1.0 / jnp.sqrt(jnp.asarray(HEAD_DIM, jnp.float32))
        scores = jnp.einsum('hgqd,hkd->hgqk', qg, k) * scale
        scores = jnp.where(m[None, None], scores, jnp.asarray(-jnp.inf, scores.dtype))
        probs = jax.nn.softmax(scores, axis=-1)
        attn = jnp.einsum('hgqk,hkd->hgqd', probs, v).reshape(HQ, R, HEAD_DIM)
        gates = jax.nn.sigmoid(gate_logits).reshape(HQ, 1, 1)
        attn = (attn * gates).transpose(1, 0, 2).reshape(R, D_MODEL)
        return attn @ Wo + bo

    return jax.jit(f)


def kernel(x, Wq, bq, Wk, bk, Wv, bv, Wo, bo, qn_w, kn_w,
           gate_logits, mask, start_pos, **_ignored):
    x = np.asarray(x, dtype=np.float32)
    Wq = np.asarray(Wq, dtype=np.float32); bq = np.asarray(bq, dtype=np.float32)
    Wk = np.asarray(Wk, dtype=np.float32); bk = np.asarray(bk, dtype=np.float32)
    Wv = np.asarray(Wv, dtype=np.float32); bv = np.asarray(bv, dtype=np.float32)
    Wo = np.asarray(Wo, dtype=np.float32); bo = np.asarray(bo, dtype=np.float32)
    qn_w = np.asarray(qn_w, dtype=np.float32); kn_w = np.asarray(kn_w, dtype=np.float32)
    gate_logits = np.asarray(gate_logits, dtype=np.float32)
    mask = np.asarray(mask)
    sp = int(np.asarray(start_pos))

    # shard spec: core c -> (batch, row block)
    shards = []
    for b in range(B):
        for blk in range(4):
            lo = blk * ROWS_PER_CORE
            hi = lo + ROWS_PER_CORE
            shards.append((b, lo, hi))

    out = np.empty((B, S, D_MODEL), dtype=np.float32)

    # Try to run on the 8 trn2 cores via jax/axon; fall back to numpy.
    # The attempt is time-bounded so kernel() can never hang on a slow or
    # wedged device compile.
    import os, signal

    class _Timeout(Exception):
        pass

    def _alarm(signum, frame):
        raise _Timeout()

    try:
        if os.environ.get("GQA_NO_DEVICE"):
            raise RuntimeError("device path disabled")
        old = signal.signal(signal.SIGALRM, _alarm)
        signal.alarm(int(os.environ.get("GQA_DEVICE_TIMEOUT", "180")))
        import jax
        devs = jax.devices()
        if len(devs) < N_CORES:
            raise RuntimeError("fewer than 8 devices")
        f = _device_fn_factory()
        results = []
        for c, (b, lo, hi) in enumerate(shards):
            d = devs[c]
            ext = hi
            pos_q = np.arange(lo, hi, dtype=np.int32) + sp
            pos_k = np.arange(ext, dtype=np.int32) + sp
            args = (x[b, lo:hi], x[b, :ext], pos_q, pos_k, Wq, bq, Wk, bk,
                    Wv, bv, Wo, bo, qn_w, kn_w, gate_logits,
                    mask[lo:hi, :ext])
            args = tuple(jax.device_put(a, d) for a in args)
            results.append((b, lo, hi, f(*args)))
        for b, lo, hi, r in results:
            out[b, lo:hi] = np.asarray(r, dtype=np.float32)
        signal.alarm(0)
        signal.signal(signal.SIGALRM, old)
        return out
    except BaseException:
        try:
            signal.alarm(0)
            signal.signal(signal.SIGALRM, old)
        except Exception:
            pass

    for (b, lo, hi) in shards:
        out[b, lo:hi] = _rows_block(
            x[b], lo, hi, Wq, bq, Wk, bk, Wv, bv, Wo, bo,
            qn_w, kn_w, gate_logits, mask, sp)
    return out

